# revision 91
# baseline (speedup 1.0000x reference)
"""Trainium2 Bass kernel for nn_AdversarialHead (scatter_memory).

Math restructuring (validated exactly against the reference):
  - fwd:  (X @ W1 + b1) @ W2 + b2 == X @ (W1@W2) + (b1@W2 + b2)   (no nonlinearity)
          fwd_in = [cur | onehot]  ->  cur @ Wc[:512] + onehot @ Wc[512:] + b'
  - inv:  softmax over the agent axis; per-type biases are constant along that
          axis so they cancel inside softmax; max-subtraction is a mathematical
          no-op (|logit| < ~4) and is skipped.

Device strategy (8 cores, data-parallel over frames; TimelineSim ~105 us/core,
close to the ~114 us/core HBM roofline for the 41 MB of per-core f32 traffic;
PE is the busiest engine at ~89 us — transposes 27, fwd matmuls 34, inv 27):
  - 256 frames (8192 rows) per core, processed in 16 supertiles of 512 rows.
  - slab row<->partition mapping (supertiles 0..14): partition p holds rows
    4p..4p+3, so every load/store moves 8KB/4KB contiguous DRAM runs per
    partition (~4x fewer DMA descriptors; SWDGE descriptor-gen, the
    head-latency driver, drops accordingly). Blocks become stride-4 row
    sets; the softmax regroups frames via a 4-dim access pattern (frame ==
    partition-group pf; reduce over (i, pa)). The LAST supertile stays
    row-major: its softmax sits on the kernel tail, where the contiguous
    layout's two-half pipeline is faster than strided 4-dim softmax APs.
  - Activations are cast f32->bf16 during the DMA load (SWDGE cast), then
    transposed on the PE (features onto partitions): per 128-row block all 8
    [128,128] transpose-mode matmuls land in one bf16 PSUM bank, evacuated
    with a single DVE copy into xT [128 feat, 8 chunks, 512 rows].
  - fwd out:   [128 rows, 256] per block, accumulated in PSUM from 4 K=128
               chunks + 1 K=18 one-hot chunk; evacuated on the ACT engine
               (plain copy; zero bias) or DVE tensor_add when bias != 0.
  - one-hot:   actions are host-replicated to [18, rows] (partition s carries
               type(s)'s global index), so ohT [18, 128] falls out of a single
               is_equal against the partition index - no transpose needed.
  - inv out:   logits.T [18, 512] per supertile via 8 accumulating matmuls
               (lhsT = Wi chunks, rhs = xT chunks), exp on ACT, group-of-32
               reduce_sum + reciprocal + normalize-multiply on DVE, stored
               transposed to DRAM; the [18, rows] -> [rows, 18] flip happens
               on the host during unshard (4.7 MB total, negligible). The
               last supertile runs its inverse head in two 256-row halves so
               the final softmax+store pipelines against matmuls (tail trim).
  - all constants ride in packed blobs (one f32, one bf16) loaded with a
    single DMA each, so no instruction accumulates too many semaphore waits.
  - softmax max-subtraction is skipped (|logit| < ~4, exact identity); the
    per-type inverse biases cancel inside softmax and are dropped.
"""

import sys

import numpy as np

if "/opt/trn_rl_repo" not in sys.path:
    sys.path.insert(0, "/opt/trn_rl_repo")

import ml_dtypes

BF16 = ml_dtypes.bfloat16

NCORES = 8
F, A, FEAT = 2048, 32, 512
FC = F // NCORES          # 256 frames per core
ROWS = FC * A             # 8192 rows per core
SUP = 512                 # rows per supertile
NSUP = ROWS // SUP        # 16
NB = SUP // 128           # 4 blocks of 128 rows
NBLK = ROWS // 128        # 64 blocks of 128 rows per core
STARTS = (0, 8, 14)
ACT_NUMS = (8, 6, 4)
S18 = 18
NOUT = 256

# f32 const blob layout (columns)
C32_BIAS = 0                   # [128, 256]
C32_PIDX = NOUT                # [128, 1] partition index 0..127
C32_W = C32_PIDX + 1           # total 257

# bf16 const blob layout (columns)
C16_WC = 0                     # [128, 4*256]  (Wc[:512] K-chunked)
C16_WI = 4 * NOUT              # [128, 8*18]   (Wi K-chunked)
C16_ID = C16_WI + 8 * S18      # [128, 128]    identity
C16_WCOH = C16_ID + 128        # [18, 256] in first 18 partitions
C16_W = C16_WCOH + NOUT        # total 1552

# set by test.py to collect a hardware profile
TRACE = False
LAST_RESULTS = None

_PROGRAM = None
_PROGRAM_KEY = None


def _build_program(use_bias):
    import concourse.mybir as mybir
    import concourse.tile as tile
    from concourse import bacc

    fp32 = mybir.dt.float32
    bf16 = mybir.dt.bfloat16
    eq = mybir.AluOpType.is_equal

    # Bacc (not raw Bass): its finalize() runs move_matmul_waits_to_ldweights
    # + generate_event_semaphores, which split multi-sem waits that the TPB
    # instruction encoding cannot carry (walrus: "Too many sync wait commands")
    nc = bacc.Bacc("TRN2", target_bir_lowering=False, debug=False)
    cur_d = nc.dram_tensor("cur", [ROWS, FEAT], fp32, kind="ExternalInput")
    nxt_d = nc.dram_tensor("nxt", [ROWS, FEAT], fp32, kind="ExternalInput")
    actr_d = nc.dram_tensor("actr", [S18, ROWS], fp32, kind="ExternalInput")
    c32_d = nc.dram_tensor("c32", [128, C32_W], fp32, kind="ExternalInput")
    c16_d = nc.dram_tensor("c16", [128, C16_W], bf16, kind="ExternalInput")
    pred_d = nc.dram_tensor("pred", [ROWS, NOUT], fp32, kind="ExternalOutput")
    invt_d = nc.dram_tensor("invt", [S18, ROWS], fp32, kind="ExternalOutput")

    with tile.TileContext(nc) as tc:
        with (
            tc.tile_pool(name="consts", bufs=1) as consts,
            tc.tile_pool(name="raw", bufs=5) as raw,
            tc.tile_pool(name="tsp", bufs=5) as tsp,
            tc.tile_pool(name="ohp", bufs=4) as ohp,
            tc.tile_pool(name="outs", bufs=5) as outs,
            tc.tile_pool(name="smp", bufs=6) as smp,
            tc.tile_pool(name="pst", bufs=4, space="PSUM") as psum_t,
            tc.tile_pool(name="psf", bufs=3, space="PSUM") as psum_f,
            tc.tile_pool(name="psi", bufs=1, space="PSUM") as psum_i,
        ):
            c32 = consts.tile([128, C32_W], fp32)
            nc.sync.dma_start(out=c32, in_=c32_d[:])
            c16 = consts.tile([128, C16_W], bf16)
            nc.sync.dma_start(out=c16, in_=c16_d[:])

            bias_sb = c32[:, C32_BIAS : C32_BIAS + NOUT]
            pidx_sb = c32[0:S18, C32_PIDX : C32_PIDX + 1]
            wc_sb = c16[:, C16_WC : C16_WC + 4 * NOUT].rearrange(
                "p (j n) -> p j n", j=4
            )
            wi_sb = c16[:, C16_WI : C16_WI + 8 * S18].rearrange(
                "p (c n) -> p c n", c=8
            )
            ident_sb = c16[:, C16_ID : C16_ID + 128]
            wcoh_sb = c16[0:S18, C16_WCOH : C16_WCOH + NOUT]

            # actions replicated (host-side) so partition s carries type(s)'s
            # global action index for every row: ohT falls out of one is_equal
            # against the partition index, already in [18, rows] orientation
            act_rep = consts.tile([S18, ROWS], fp32)
            nc.sync.dma_start(out=act_rep, in_=actr_d[:])

            for s in range(NSUP):
                r0 = s * SUP
                # slab mapping: partition p holds rows 4p..4p+3, so each
                # partition's DMA data is one contiguous 8KB DRAM run (vs 4
                # scattered 2KB rows) - far fewer descriptors per transfer.
                # The LAST supertile stays row-major: its softmax sits on the
                # kernel tail, where the contiguous layout's half-split
                # pipeline is faster than slab's strided 4-dim softmax APs.
                slab = s < NSUP - 1
                ld = "(p i) c -> p i c" if slab else "(i p) c -> p i c"
                cur_sb = raw.tile([128, NB, FEAT], bf16, tag="cur")
                nxt_sb = raw.tile([128, NB, FEAT], bf16, tag="nxt")
                nc.gpsimd.dma_start(
                    out=cur_sb,
                    in_=cur_d[r0 : r0 + SUP, :].rearrange(ld, i=NB),
                )
                nc.gpsimd.dma_start(
                    out=nxt_sb,
                    in_=nxt_d[r0 : r0 + SUP, :].rearrange(ld, i=NB),
                )

                # xT chunks 0-3 = cur.T, 4-7 = nxt.T (matches wi row order)
                xT = tsp.tile([128, 8, SUP], bf16, tag="xT")
                fwd_sb = outs.tile([128, NB, NOUT], fp32, tag="fwd")

                for b in range(NB):
                    bsl = slice(b * 128, (b + 1) * 128)
                    # all 8 feature-chunk transposes of this block go into one
                    # PSUM bank (bf16), evacuated with a single DVE copy
                    pt = psum_t.tile([128, 8, 128], bf16, tag="pt")
                    if s < 2 and b <= 1:
                        # early blocks: evacuate the cur half on its own so
                        # the PE+DVE pipeline runs during nxt's transfer
                        # (loads still race compute in the first supertiles)
                        for j in range(4):
                            nc.tensor.transpose(
                                pt[:, j, :],
                                cur_sb[:, b, j * 128 : (j + 1) * 128],
                                ident_sb,
                            )
                        nc.vector.tensor_copy(xT[:, 0:4, bsl], pt[:, 0:4, :])
                        for j in range(4):
                            nc.tensor.transpose(
                                pt[:, 4 + j, :],
                                nxt_sb[:, b, j * 128 : (j + 1) * 128],
                                ident_sb,
                            )
                        nc.vector.tensor_copy(xT[:, 4:8, bsl], pt[:, 4:8, :])
                    else:
                        for j in range(4):
                            nc.tensor.transpose(
                                pt[:, j, :],
                                cur_sb[:, b, j * 128 : (j + 1) * 128],
                                ident_sb,
                            )
                            nc.tensor.transpose(
                                pt[:, 4 + j, :],
                                nxt_sb[:, b, j * 128 : (j + 1) * 128],
                                ident_sb,
                            )
                        nc.vector.tensor_copy(xT[:, :, bsl], pt)

                    # block b holds rows {4p + b} (slab) or {b*128 + p}
                    ohT = ohp.tile([S18, 128], bf16, tag="ohT")
                    nc.vector.tensor_scalar(
                        out=ohT,
                        in0=act_rep[:, r0 : r0 + SUP].rearrange(
                            "s (p i) -> s i p" if slab else "s (i p) -> s i p",
                            i=NB,
                        )[:, b, :],
                        scalar1=pidx_sb,
                        scalar2=None,
                        op0=eq,
                    )

                    pf = psum_f.tile([128, NOUT], fp32, tag="pf")
                    for j in range(4):
                        nc.tensor.matmul(
                            pf, xT[:, j, bsl], wc_sb[:, j, :],
                            start=(j == 0), stop=False,
                        )
                    nc.tensor.matmul(pf, ohT, wcoh_sb, start=False, stop=True)
                    if use_bias:
                        nc.vector.tensor_add(fwd_sb[:, b, :], pf, bias_sb)
                    else:
                        nc.scalar.activation(
                            out=fwd_sb[:, b, :], in_=pf,
                            func=mybir.ActivationFunctionType.Copy,
                        )

                nc.sync.dma_start(
                    out=pred_d[r0 : r0 + SUP, :].rearrange(ld, i=NB),
                    in_=fwd_sb,
                )

                pi = psum_i.tile([S18, SUP], fp32, tag="pi")
                if slab:
                    # inverse head: logits.T [18, 512] for the supertile.
                    # Under slab, logit column (i*128 + pf*8 + pa) is row
                    # (32*pf + 4*pa + i): frame == pf, so softmax reduces
                    # the (i, pa) sub-axes.
                    for c in range(8):
                        nc.tensor.matmul(
                            pi, wi_sb[:, c, :], xT[:, c, :],
                            start=(c == 0), stop=(c == 7),
                        )
                    ngf = SUP // A
                    el = smp.tile([S18, ngf, NB, 8], fp32, tag="el")
                    nc.scalar.activation(
                        out=el,
                        in_=pi[:].rearrange(
                            "s (i pf pa) -> s pf i pa", i=NB, pa=8
                        ),
                        func=mybir.ActivationFunctionType.Exp,
                    )
                    sm = smp.tile([S18, ngf], fp32, tag="sm")
                    nc.vector.reduce_sum(
                        out=sm, in_=el, axis=mybir.AxisListType.XY
                    )
                    nc.vector.reciprocal(sm, sm)
                    # laid out in store order (pf, pa, i): the DVE multiply
                    # handles the 4-dim permuted write; the DMA then sees a
                    # plain contiguous transfer on both sides
                    inv_sb = outs.tile([S18, ngf, 8, NB], fp32, tag="inv")
                    nc.vector.tensor_mul(
                        inv_sb[:].rearrange("s pf pa i -> s pf i pa"), el,
                        sm[:, :, None, None].to_broadcast((S18, ngf, NB, 8)),
                    )
                    nc.sync.dma_start(
                        out=invt_d[:, r0 : r0 + SUP], in_=inv_sb
                    )
                else:
                    # last supertile (row-major): two contiguous halves so
                    # the final softmax+store pipelines against the second
                    # half's matmuls (shorter kernel tail)
                    hw_ = SUP // 2
                    ngf = hw_ // A
                    for h in range(2):
                        csl = slice(h * hw_, (h + 1) * hw_)
                        for c in range(8):
                            nc.tensor.matmul(
                                pi[:, csl], wi_sb[:, c, :], xT[:, c, csl],
                                start=(c == 0), stop=(c == 7),
                            )
                        el = smp.tile([S18, ngf, A], fp32, tag="el2")
                        nc.scalar.activation(
                            out=el,
                            in_=pi[:, csl].rearrange("p (g a) -> p g a", a=A),
                            func=mybir.ActivationFunctionType.Exp,
                        )
                        sm = smp.tile([S18, ngf], fp32, tag="sm2")
                        nc.vector.reduce_sum(
                            out=sm, in_=el, axis=mybir.AxisListType.X
                        )
                        nc.vector.reciprocal(sm, sm)
                        inv_sb = outs.tile([S18, ngf, A], fp32, tag="inv2")
                        nc.vector.tensor_mul(
                            inv_sb, el,
                            sm[:, :, None].to_broadcast((S18, ngf, A)),
                        )
                        nc.sync.dma_start(
                            out=invt_d[:, r0 + h * hw_ : r0 + (h + 1) * hw_],
                            in_=inv_sb,
                        )

    nc.finalize()
    return nc


def _get_program(use_bias=False):
    global _PROGRAM, _PROGRAM_KEY
    if _PROGRAM is None or _PROGRAM_KEY != use_bias:
        _PROGRAM = _build_program(use_bias)
        _PROGRAM_KEY = use_bias
    return _PROGRAM


def _pack_consts(W1, b1, W2, b2, Wi0, Wi1, Wi2):
    W1 = np.asarray(W1, dtype=np.float64)
    W2 = np.asarray(W2, dtype=np.float64)
    b1 = np.asarray(b1, dtype=np.float64)
    b2 = np.asarray(b2, dtype=np.float64)

    wc_full = W1 @ W2                      # [530, 256]
    bvec = (b1 @ W2 + b2).astype(np.float32)

    c32_shared = np.zeros((128, C32_W), np.float32)
    c32_shared[:, C32_BIAS : C32_BIAS + NOUT] = bvec
    c32_shared[:, C32_PIDX] = np.arange(128, dtype=np.float32)

    c16 = np.zeros((128, C16_W), BF16)
    wc = wc_full[:FEAT].astype(np.float32).astype(BF16)       # [512, 256]
    c16[:, C16_WC : C16_WC + 4 * NOUT] = (
        wc.reshape(4, 128, NOUT).transpose(1, 0, 2).reshape(128, 4 * NOUT)
    )
    wi = np.concatenate(
        [np.asarray(Wi0), np.asarray(Wi1), np.asarray(Wi2)], axis=1
    ).astype(BF16)                                            # [1024, 18]
    c16[:, C16_WI : C16_WI + 8 * S18] = (
        wi.reshape(8, 128, S18).transpose(1, 0, 2).reshape(128, 8 * S18)
    )
    c16[:, C16_ID : C16_ID + 128] = np.eye(128, dtype=np.float32).astype(BF16)
    c16[:S18, C16_WCOH : C16_WCOH + NOUT] = (
        wc_full[FEAT:].astype(np.float32).astype(BF16)
    )
    return c32_shared, c16


def kernel(
    current_feature, next_feature, actions,
    W1, b1, W2, b2, Wi0, bi0, Wi1, bi1, Wi2, bi2,
):
    global LAST_RESULTS
    from concourse.bass_utils import run_bass_kernel_spmd

    cur = np.ascontiguousarray(np.asarray(current_feature, dtype=np.float32))
    nxt = np.ascontiguousarray(np.asarray(next_feature, dtype=np.float32))
    act = np.asarray(actions)

    c32_shared, c16 = _pack_consts(W1, b1, W2, b2, Wi0, Wi1, Wi2)

    starts = np.asarray(STARTS, dtype=act.dtype)
    act_g = (act + starts[None, :, None]).astype(np.float32)  # [F, 3, A]
    # replicated layout [18, rows]: row s = type(s)'s global action index
    ty_of_s = np.repeat(np.arange(3), ACT_NUMS)               # [18]
    actr = (
        act_g.reshape(NCORES, FC, 3, A)
        .transpose(0, 2, 1, 3)
        .reshape(NCORES, 3, ROWS)[:, ty_of_s, :]
    )                                                          # [NC, 18, rows]

    cur = cur.reshape(NCORES, ROWS, FEAT)
    nxt = nxt.reshape(NCORES, ROWS, FEAT)

    in_maps = [
        {
            "cur": cur[i],
            "nxt": nxt[i],
            "actr": np.ascontiguousarray(actr[i]),
            "c32": c32_shared,
            "c16": c16,
        }
        for i in range(NCORES)
    ]

    use_bias = bool(np.any(c32_shared[0, C32_BIAS : C32_BIAS + NOUT] != 0.0))
    nc = _get_program(use_bias)
    res = run_bass_kernel_spmd(nc, in_maps, list(range(NCORES)), trace=TRACE)
    LAST_RESULTS = res

    pred = np.concatenate([r["pred"] for r in res.results], axis=0)
    pred = pred.reshape(F, A, NOUT)
    inv = np.concatenate(
        [np.ascontiguousarray(r["invt"].T) for r in res.results], axis=0
    ).reshape(F, A, S18)
    a0 = np.ascontiguousarray(inv[:, :, 0:8])
    a1 = np.ascontiguousarray(inv[:, :, 8:14])
    a2 = np.ascontiguousarray(inv[:, :, 14:18])
    return (pred, a0, a1, a2)
